# revision 19
# baseline (speedup 1.0000x reference)
"""Trainium2 Bass kernel for nn_FLASH_ShareA_FFConvM — fp8 DoubleRow version.

Data-parallel over (batch, seq-half): 8 cores x 4096 tokens (16 chunks of 256).
Weights replicated. Big matmuls (hidden/vgate/gate, attention-V, wcomb, w_out)
run in fp8e4 with DoubleRow perf mode (2x PE throughput); qk and sim stay bf16.

Per core:
  phase A: LN stats (bn_stats/bn_aggr, rstd via DVE pow), normalized bf16 x
    kept token-major (xs_bf, residual source) and channel-major via XBAR DMA
    transpose (xsT bf16 for qk; xsT8 fp8 cast on gpsimd for fp8 matmuls).
    Token-shift of channels 0..255 = column offset on xsT c-chunks 0..1.
  phase B per chunk pair: qk^T (bf16) -> per-chunk OffsetScale (host
    pre-scales q-side by HD^-0.25... actually by S so sim psum is pre-scaled)
    -> q/k/qs/ks bf16 -> sim (bf16, masked-block skipped) -> DVE mask-add into
    a gapped tmp -> one ACT Exp -> expt8 fp8 (= exp/16) -> denominator via
    fp8 ones-matmul -> reciprocal -> gpsimd partition_broadcast -> attn8
    (= 16*attn) on gpsimd -> V matmul fp8 DR -> silu (scale 1/128) -> *gate
    (= 16*gate, folded og scale) -> og8 -> fin psum = xs8@wcomb8 + og8@woa8
    (both 256x scaled) -> y = fin/256*silu(vgate) + xs (bf16 out).
"""

import sys

sys.path.insert(0, "/opt/trn_rl_repo")

import numpy as np
import ml_dtypes
from contextlib import ExitStack

import concourse.bass as bass
import concourse.tile as tile
from concourse import bacc, mybir

F32 = mybir.dt.float32
BF16 = mybir.dt.bfloat16
F8 = mybir.dt.float8e4
AX = mybir.AxisListType
ALU = mybir.AluOpType
ACTF = mybir.ActivationFunctionType
DR = mybir.MatmulPerfMode.DoubleRow

B, SEQ, DIM = 4, 8192, 512
G, QK = 32, 128
CHUNK = SEQ // G          # 256 tokens per attention chunk
HD = QK // 4              # 32
SCALE = float(HD) ** -0.5
HID = DIM                 # 512
EPS = 1e-5
N_CORES = 8
T_CORE = SEQ // 2         # 4096 tokens per core
NEG = -1.0e30
LN16 = float(np.log(16.0))

BF = ml_dtypes.bfloat16
F8NP = ml_dtypes.float8_e4m3


def build_core_program(ctx: ExitStack, tc, aps, n_tok, apply_g, apply_b):
    nc = tc.nc
    n_tiles = n_tok // 128            # 32 (excl. halo tile)
    n_chunks = n_tok // CHUNK
    NT = n_tiles + 1                  # 33 tiles incl halo tile 0
    n_pad = n_tok + 128

    xp = aps["xp"]; yout = aps["y"]

    consts = ctx.enter_context(tc.tile_pool(name="consts", bufs=1))
    persist = ctx.enter_context(tc.tile_pool(name="persist", bufs=1))
    work = ctx.enter_context(tc.tile_pool(name="work", bufs=1))
    psum = ctx.enter_context(tc.tile_pool(name="psum", bufs=1, space="PSUM"))

    def cload(name, shape, dtype):
        if shape[0] > 128:
            k = shape[0] // 128
            t = consts.tile([128, k, shape[1]], dtype, name=f"c_{name}",
                            tag=f"c_{name}")
            nc.sync.dma_start(t[:], aps[name].rearrange("(k p) c -> p k c",
                                                        p=128))
        else:
            t = consts.tile(shape, dtype, name=f"c_{name}", tag=f"c_{name}")
            nc.sync.dma_start(t[:], aps[name])
        return t

    wqk = cload("wqk", [512, 128], BF16)        # [128, 4, 128]
    whvg8 = cload("whvg8", [512, 1024], F8)     # [hid | vgate] * 16
    wga8 = cload("wga8", [512, 512], F8)        # gate * 16
    wcomb8 = cload("wcomb8", [512, 512], F8)    # (w_hid @ w_out[:512]) * 256
    woa8 = cload("woa8", [2048, 512], F8)       # w_out[512:] * 16
    g46 = cload("g46", [128, 8], F32)
    b46 = cload("b46", [128, 8], F32)
    ltri = cload("ltri", [128, 128], BF16)
    negi = cload("negi", [128, 128], BF16)
    negrow = cload("negrow", [1, 128], BF16)
    onesrow = cload("onesrow", [1, 128], BF16)
    if apply_g:
        lng = cload("lng", [128, 512], F32)
    if apply_b:
        lnb = cload("lnb", [128, 512], F32)

    ones16 = consts.tile([128, 1], F8, name="ones16", tag="ones16")
    nc.vector.memset(ones16[:], 1.0 / 16.0)
    bln16 = consts.tile([128, 1], F32, name="bln16", tag="bln16")
    nc.vector.memset(bln16[:], -LN16)
    epsb = consts.tile([128, 1], F32, name="epsb", tag="epsb")
    nc.vector.memset(epsb[:], EPS)

    xsT = persist.tile([128, 4, n_pad], BF16, name="xsT", tag="xsT")
    xsT8 = persist.tile([128, 4, n_pad], F8, name="xsT8", tag="xsT8")
    xs_bf = persist.tile([128, NT, 512], BF16, name="xs_bf", tag="xs_bf")
    stats = persist.tile([128, NT, 2], F32, name="stats", tag="stats")

    def act(bi):
        tc.chain_iter_dep("actfn", bi.ins)
        return bi

    # ---------------- phase A1: LN stats ----------------
    stats_next = 0
    rstd_hi = 0

    def emit_stats(upto):
        nonlocal stats_next
        while stats_next <= min(upto, n_tiles):
            i0 = stats_next
            k = min(2, NT - i0)
            x2 = work.tile([128, k, 512], F32, name=f"xst{i0}", tag="xst",
                           bufs=4)
            nc.sync.dma_start(
                x2[:], xp[i0 * 128:(i0 + k) * 128, :].rearrange(
                    "(k p) c -> p k c", p=128))
            for j in range(k):
                bns = work.tile([128, 6], F32, name=f"bns{i0}_{j}", tag="bns",
                                bufs=2)
                nc.vector.bn_stats(out=bns[:], in_=x2[:, j, :])
                nc.vector.bn_aggr(out=stats[:, i0 + j, :], in_=bns[:])
            stats_next += k

    def ensure_stats(upto):
        nonlocal rstd_hi
        emit_stats(upto)
        if stats_next > rstd_hi:
            # rstd = 1/sqrt(var + eps): batched ACT Sqrt + DVE reciprocal
            sl = stats[:, rstd_hi:stats_next, 1:2]
            act(nc.scalar.activation(out=sl, in_=sl, func=ACTF.Sqrt,
                                     bias=epsb[:]))
            nc.vector.reciprocal(out=sl, in_=sl)
            rstd_hi = stats_next

    # ---------------- phase A2: normalize + transpose + fp8 cast ----------
    ln_next = 0

    def emit_ln(upto):
        nonlocal ln_next
        while ln_next <= min(upto, n_tiles):
            i = ln_next
            x_t = work.tile([128, 512], F32, name=f"xn{i}", tag="xn", bufs=4)
            nc.sync.dma_start(x_t[:], xp[i * 128:(i + 1) * 128, :])
            if apply_g or apply_b:
                xf = work.tile([128, 512], F32, name=f"xf{i}", tag="xf",
                               bufs=2)
                nc.vector.tensor_scalar(
                    out=xf[:], in0=x_t[:], scalar1=stats[:, i, 0:1],
                    scalar2=stats[:, i, 1:2], op0=ALU.subtract, op1=ALU.mult)
                if apply_g:
                    nc.vector.tensor_tensor(out=(xf[:] if apply_b else
                                                 xs_bf[:, i, :]),
                                            in0=xf[:], in1=lng[:],
                                            op=ALU.mult)
                if apply_b:
                    nc.vector.tensor_tensor(out=xs_bf[:, i, :], in0=xf[:],
                                            in1=lnb[:], op=ALU.add)
            else:
                nc.vector.tensor_scalar(
                    out=xs_bf[:, i, :], in0=x_t[:], scalar1=stats[:, i, 0:1],
                    scalar2=stats[:, i, 1:2], op0=ALU.subtract, op1=ALU.mult)
            nc.sync.dma_start(xsT[:, :, i * 128:(i + 1) * 128],
                              xs_bf[:, i, :], transpose=True)
            # xsT8 carries the token-shift for c-chunks 0..1 baked in, so
            # every fp8 matmul AP stays 128-aligned (dual-fp8 LW restriction)
            c0 = i * 128
            if i == 0:
                nc.vector.memset(xsT8[:, 0:2, 0:1], 0.0)
                nc.vector.tensor_copy(out=xsT8[:, 0:2, 1:128],
                                      in_=xsT[:, 0:2, 0:127])
            else:
                nc.vector.tensor_copy(out=xsT8[:, 0:2, c0:c0 + 128],
                                      in_=xsT[:, 0:2, c0 - 1:c0 + 127])
            nc.vector.tensor_copy(out=xsT8[:, 2:4, c0:c0 + 128],
                                  in_=xsT[:, 2:4, c0:c0 + 128])
            ln_next += 1

    def xshB(cc, col0, width):
        c0 = col0 - 1 if cc < 2 else col0
        return xsT[:, cc, c0:c0 + width]

    def xs8p(pair, col0, width):
        return xsT8[:, 2 * pair:2 * pair + 2, col0:col0 + width]

    ensure_stats(9)
    emit_ln(5)

    # ---------------- phase B ----------------
    for gp in range(n_chunks // 2):
        if gp == 0:
            ensure_stats(min(22, n_tiles))
        elif gp == 1:
            ensure_stats(n_tiles)
        emit_ln(4 * gp + 6)
        colP = 128 + gp * 2 * CHUNK

        # qk^T for the pair (bf16)
        qkps = psum.tile([128, 512], F32, name=f"qkps{gp}", tag="mmA", bufs=2)
        for cc in range(4):
            nc.tensor.matmul(qkps[:], wqk[:, cc, :], xshB(cc, colP, 512),
                             start=(cc == 0), stop=(cc == 3))
        qkT = work.tile([128, 512], BF16, name=f"qkT{gp}", tag="qkT", bufs=2)
        nc.vector.tensor_copy(out=qkT[:], in_=qkps[:])

        # gate^T for the pair (fp8 DR) -> bf16 = 16*gate
        gate_bf = []
        for ee in range(4):
            gps = psum.tile([128, 512], F32, name=f"g{gp}_{ee}", tag="mmA",
                            bufs=2)
            for th in range(2):
                for pair in range(2):
                    nc.tensor.matmul(
                        gps[:, th * 256:(th + 1) * 256],
                        wga8[:, 2 * pair:2 * pair + 2,
                             ee * 128:(ee + 1) * 128],
                        xs8p(pair, colP + th * 256, 256),
                        start=(pair == 0), stop=(pair == 1), perf_mode=DR)
            gb = work.tile([128, 512], BF16, name=f"gate{gp}_{ee}",
                           tag=f"gate{ee}", bufs=2)
            act(nc.scalar.activation(out=gb[:], in_=gps[:], func=ACTF.Copy))
            gate_bf.append(gb)

        chunk_state = []
        for g in (2 * gp, 2 * gp + 1):
            half = g % 2
            colU = 128 + g * CHUNK
            qk_c = qkT[:, half * 256:(half + 1) * 256]

            qsum = work.tile([128, 1], F32, name=f"qsum{g}", tag="qsum",
                             bufs=2)
            nc.vector.tensor_reduce(out=qsum[:], in_=qk_c, axis=AX.X,
                                    op=ALU.add)
            offs = work.tile([128, 8], F32, name=f"offs{g}", tag="offs",
                             bufs=2)
            nc.vector.scalar_tensor_tensor(out=offs[:], in0=g46[:],
                                           scalar=qsum[:], in1=b46[:],
                                           op0=ALU.mult, op1=ALU.add)
            # cols: 0 qoff*S, 1 koff, 2 qsc*S, 3 ksc, 4 qsoff*S, 5 ksoff,
            #       6 qsc, 7 unused
            qT = work.tile([128, 256], BF16, name=f"qT{g}", tag="qT", bufs=3)
            kT = work.tile([128, 256], BF16, name=f"kT{g}", tag="kT", bufs=3)
            qsT = work.tile([128, 256], BF16, name=f"qsT{g}", tag="qsT",
                            bufs=3)
            ksT = work.tile([128, 256], BF16, name=f"ksT{g}", tag="ksT",
                            bufs=3)
            nc.vector.tensor_scalar(out=qT[:], in0=qk_c,
                                    scalar1=offs[:, 2:3], scalar2=offs[:, 0:1],
                                    op0=ALU.mult, op1=ALU.add)
            nc.vector.tensor_scalar(out=kT[:], in0=qk_c,
                                    scalar1=offs[:, 3:4], scalar2=offs[:, 1:2],
                                    op0=ALU.mult, op1=ALU.add)
            nc.vector.tensor_copy(out=qsT[:, 0:1], in_=offs[:, 4:5])
            nc.vector.tensor_scalar(out=qsT[:, 1:256], in0=qT[:, 0:255],
                                    scalar1=offs[:, 6:7], scalar2=offs[:, 4:5],
                                    op0=ALU.mult, op1=ALU.add)
            nc.vector.tensor_copy(out=ksT[:, 0:1], in_=offs[:, 5:6])
            nc.vector.tensor_scalar(out=ksT[:, 1:256], in0=kT[:, 0:255],
                                    scalar1=offs[:, 3:4], scalar2=offs[:, 5:6],
                                    op0=ALU.mult, op1=ALU.add)

            # sim (bf16) + causal mask via PE (ltri x negi) + -inf gap
            # expt8 free layout per head: [jt0 i0..255 | gap | jt1 i128..255]
            expt8 = work.tile([128, 4, 512], F8, name=f"expt{g}", tag="expt",
                              bufs=3)
            attn8 = work.tile([128, 4, 512], F8, name=f"attn{g}", tag="attn",
                              bufs=3)
            if g < 3:
                nc.gpsimd.memset(attn8[:, :, 256:384], 0.0)
            for h in range(4):
                Qt = qT if h < 2 else qsT
                Kt = kT if h < 2 else ksT
                dr = (h % 2) * 64
                simps = psum.tile([128, 512], F32, name=f"sim{g}_{h}",
                                  tag="sim", bufs=2)
                nc.tensor.matmul(simps[:, 0:128], Kt[dr:dr + 64, 0:128],
                                 Qt[dr:dr + 64, 0:128], start=True, stop=False)
                nc.tensor.matmul(simps[:, 0:128], ltri[:], negi[:],
                                 start=False, stop=True)
                nc.tensor.matmul(simps[:, 128:256], Kt[dr:dr + 64, 0:128],
                                 Qt[dr:dr + 64, 128:256], start=True,
                                 stop=True)
                nc.tensor.matmul(simps[:, 384:512], Kt[dr:dr + 64, 128:256],
                                 Qt[dr:dr + 64, 128:256], start=True,
                                 stop=False)
                nc.tensor.matmul(simps[:, 384:512], ltri[:], negi[:],
                                 start=False, stop=True)
                nc.tensor.matmul(simps[:, 256:384], onesrow[:], negrow[:],
                                 start=True, stop=True)
                act(nc.scalar.activation(out=expt8[:, h, :], in_=simps[:],
                                         func=ACTF.Exp, bias=bln16[:]))

            # denominator: fp8 DR ones-matmul -> [1, 256] per head
            e4 = expt8.rearrange("p h (k i) -> p h k i", k=2)
            a4 = attn8.rearrange("p h (k i) -> p h k i", k=2)
            denps = []
            for hh in range(2):
                dps = psum.tile([128, 512], F32, name=f"den{g}_{hh}",
                                tag="sim", bufs=2)
                for j in range(2):
                    h = hh * 2 + j
                    for kk in range(2):
                        nc.tensor.matmul(
                            dps[0:1, j * 256:(j + 1) * 256], ones16[:],
                            expt8[:, h, kk * 256:(kk + 1) * 256],
                            start=(kk == 0), stop=(kk == 1))
                denps.append(dps)
            rec = work.tile([1, 1024], F32, name=f"rec{g}", tag="rec", bufs=3)
            for hh in range(2):
                nc.vector.reciprocal_approx_fast(
                    out=rec[:, hh * 512:(hh + 1) * 512],
                    in_=denps[hh][0:1, :])
            recb = work.tile([128, 1024], F32, name=f"recb{g}", tag="recb",
                             bufs=3)
            nc.gpsimd.partition_broadcast(recb[:], rec[:], channels=128)
            r4 = recb.rearrange("p (h i) -> p h i", h=4)
            nc.gpsimd.tensor_tensor(out=attn8[:, :, 0:256],
                                    in0=expt8[:, :, 0:256],
                                    in1=r4[:, :, 0:256], op=ALU.mult)
            nc.gpsimd.tensor_tensor(out=attn8[:, :, 384:512],
                                    in0=expt8[:, :, 384:512],
                                    in1=r4[:, :, 128:256], op=ALU.mult)

            chunk_state.append((g, half, colU, a4))

        hv_state = []
        for (g, half, colU, a4) in chunk_state:
            # hidden + vgate (fp8 DR)
            hid8 = work.tile([128, 2, 512], F8, name=f"hid8_{g}", tag="hid8",
                             bufs=3)
            svg = work.tile([128, 2, 512], BF16, name=f"svg{g}", tag="svg",
                            bufs=3)
            for tt in range(2):
                colT = colU + tt * 128
                hps = psum.tile([128, 512], F32, name=f"h{g}_{tt}", tag="hv",
                                bufs=2)
                vps2 = psum.tile([128, 512], F32, name=f"v{g}_{tt}", tag="hv",
                                 bufs=2)
                for nh in range(2):
                    for pair in range(2):
                        nc.tensor.matmul(
                            hps[:, nh * 256:(nh + 1) * 256],
                            xs8p(pair, colT, 128),
                            whvg8[:, 2 * pair:2 * pair + 2,
                                  nh * 256:(nh + 1) * 256],
                            start=(pair == 0), stop=(pair == 1), perf_mode=DR)
                for nh in range(2):
                    for pair in range(2):
                        nc.tensor.matmul(
                            vps2[:, nh * 256:(nh + 1) * 256],
                            xs8p(pair, colT, 128),
                            whvg8[:, 2 * pair:2 * pair + 2,
                                  512 + nh * 256:512 + (nh + 1) * 256],
                            start=(pair == 0), stop=(pair == 1), perf_mode=DR)
                act(nc.scalar.activation(out=hid8[:, tt, :], in_=hps[:],
                                         func=ACTF.Copy, scale=0.5))
                act(nc.scalar.activation(out=svg[:, tt, :], in_=vps2[:],
                                         func=ACTF.Silu, scale=1.0 / 16.0))
            hv_state.append((g, half, colU, a4, hid8, svg))

        # V matmul + silu + gate -> og8 (= 16*og), then fin + y, per chunk
        for (g, half, colU, a4, hid8, svg) in hv_state:
            og8 = work.tile([128, 4, 1024], F8, name=f"og8_{g}", tag="og8",
                            bufs=3)
            for ee in range(4):
                for hh in range(2):
                    vps = psum.tile([128, 512], F32, name=f"vo{g}_{ee}_{hh}",
                                    tag="vf", bufs=2)
                    for j in range(2):
                        nc.tensor.matmul(
                            vps[:, j * 256:(j + 1) * 256],
                            hid8[:, :, ee * 128:(ee + 1) * 128],
                            a4[:, hh * 2 + j], start=True, stop=True,
                            perf_mode=DR)
                    osl = work.tile([128, 512], BF16, name=f"osl{g}{ee}{hh}",
                                    tag="osl", bufs=4)
                    act(nc.scalar.activation(out=osl[:], in_=vps[:],
                                             func=ACTF.Silu,
                                             scale=1.0 / 128.0))
                    gsl = gate_bf[ee][:, half * 256:(half + 1) * 256]
                    gbc = gsl.unsqueeze(1).broadcast_to((128, 2, 256))
                    nc.vector.tensor_tensor(
                        out=og8[:, ee, hh * 512:(hh + 1) * 512].rearrange(
                            "p (h i) -> p h i", h=2),
                        in0=osl.rearrange("p (h i) -> p h i", h=2),
                        in1=gbc, op=ALU.mult)

            for tt in range(2):
                colT = colU + tt * 128
                u = g * 2 + tt + 1        # xp/xs_bf tile index for this y tile
                fps = psum.tile([128, 512], F32, name=f"f{g}_{tt}", tag="vf",
                                bufs=2)
                for dh in range(2):
                    for pair in range(2):
                        nc.tensor.matmul(
                            fps[:, dh * 256:(dh + 1) * 256],
                            xs8p(pair, colT, 128),
                            wcomb8[:, 2 * pair:2 * pair + 2,
                                   dh * 256:(dh + 1) * 256],
                            start=(pair == 0), stop=False, perf_mode=DR)
                    for h in range(4):
                        for j in range(2):
                            ff0 = h * 4 + 2 * j
                            nc.tensor.matmul(
                                fps[:, dh * 256:(dh + 1) * 256],
                                og8[:, 2 * j:2 * j + 2,
                                    h * 256 + tt * 128:
                                    h * 256 + tt * 128 + 128],
                                woa8[:, ff0:ff0 + 2,
                                     dh * 256:(dh + 1) * 256],
                                start=False, stop=(h == 3 and j == 1),
                                perf_mode=DR)
                y_bf = work.tile([128, 512], BF16, name=f"y{g}_{tt}", tag="y",
                                 bufs=3)
                nc.vector.scalar_tensor_tensor(out=y_bf[:], in0=fps[:],
                                               scalar=1.0 / 256.0,
                                               in1=svg[:, tt, :],
                                               op0=ALU.mult, op1=ALU.mult)
                xsp = work.tile([128, 256], BF16, name=f"xsp{g}_{tt}",
                                tag="xsp", bufs=2)
                nc.sync.dma_start(xsp[1:128, :], xs_bf[0:127, u, 0:256])
                nc.sync.dma_start(xsp[0:1, :], xs_bf[127:128, u - 1, 0:256])
                nc.vector.tensor_tensor(out=y_bf[:, 0:256],
                                        in0=y_bf[:, 0:256], in1=xsp[:],
                                        op=ALU.add)
                nc.vector.tensor_tensor(out=y_bf[:, 256:512],
                                        in0=y_bf[:, 256:512],
                                        in1=xs_bf[:, u, 256:512], op=ALU.add)
                nc.sync.dma_start(yout[(u - 1) * 128:u * 128, :], y_bf[:])


def make_host_inputs(x, ln_g, ln_b, w_qk, g4, b4, g2, b2, w_hidden, w_gate,
                     w_out, n_tok=T_CORE):
    x = np.asarray(x, np.float32)
    ln_g = np.asarray(ln_g, np.float32)
    ln_b = np.asarray(ln_b, np.float32)
    apply_g = not np.all(ln_g == 1.0)
    apply_b = bool(np.any(ln_b != 0.0))

    w_hidden = np.asarray(w_hidden, np.float32)
    w_out = np.asarray(w_out, np.float32)
    w_gate = np.asarray(w_gate, np.float32)
    w_qk = np.asarray(w_qk, np.float32)
    g4 = np.asarray(g4, np.float32)
    b4 = np.asarray(b4, np.float32)
    g2 = np.asarray(g2, np.float32)
    b2 = np.asarray(b2, np.float32)

    wcomb = w_hidden[:, :HID] @ w_out[:HID, :]

    S = SCALE
    # g46/b46 cols: qoff*S, koff, qsc*S, ksc, qsoff*S, ksoff, qsc, 0
    gcols = [g4[0] * S, g4[1], g4[2] * S, g4[3], g2[0] * S, g2[1], g4[2],
             np.zeros(QK, np.float32)]
    bcols = [b4[0] * S, b4[1], b4[2] * S, b4[3], b2[0] * S, b2[1], b4[2],
             np.zeros(QK, np.float32)]
    g46 = (np.stack(gcols, axis=1) / CHUNK).astype(np.float32).copy()
    b46 = np.stack(bcols, axis=1).astype(np.float32).copy()

    kk_, jj_ = np.meshgrid(np.arange(128), np.arange(128), indexing="ij")
    ltri = (kk_ < jj_).astype(np.float32)          # [k, j] = 1 if k < j
    negi = (np.eye(128) * NEG).astype(np.float32)  # [k, i] = NEG * delta
    negrow = np.full((1, 128), NEG, np.float32)
    onesrow = np.ones((1, 128), np.float32)

    shared = {
        "wqk": w_qk.astype(BF),
        "whvg8": (np.concatenate([w_hidden[:, :HID], w_gate], axis=1)
                  * 16.0).astype(F8NP),
        "wga8": (w_hidden[:, HID:] * 16.0).astype(F8NP),
        "wcomb8": (wcomb * 256.0).astype(F8NP),
        "woa8": (w_out[HID:, :] * 16.0).astype(F8NP),
        "g46": g46,
        "b46": b46,
        "ltri": ltri.astype(BF),
        "negi": negi.astype(BF),
        "negrow": negrow.astype(BF),
        "onesrow": onesrow.astype(BF),
    }
    if apply_g:
        shared["lng"] = np.broadcast_to(ln_g, (128, DIM)).copy()
    if apply_b:
        shared["lnb"] = np.broadcast_to(ln_b, (128, DIM)).copy()

    n_half = x.shape[1] // n_tok
    per_core = []
    for core in range(x.shape[0] * n_half):
        b = core // n_half
        h = core % n_half
        t0 = h * n_tok
        xp = np.zeros((n_tok + 128, DIM), np.float32)
        xp[128:] = x[b, t0:t0 + n_tok]
        if t0 > 0:
            xp[127] = x[b, t0 - 1]
        per_core.append({"xp": xp})
    return shared, per_core, apply_g, apply_b


def build_bass(n_tok, apply_g, apply_b):
    nc = bacc.Bacc("TRN2", target_bir_lowering=False, debug=False,
                   num_devices=1)
    specs = {
        "xp": ([n_tok + 128, DIM], F32),
        "wqk": ([512, 128], BF16),
        "whvg8": ([512, 1024], F8),
        "wga8": ([512, 512], F8),
        "wcomb8": ([512, 512], F8),
        "woa8": ([2048, 512], F8),
        "g46": ([128, 8], F32),
        "b46": ([128, 8], F32),
        "ltri": ([128, 128], BF16),
        "negi": ([128, 128], BF16),
        "negrow": ([1, 128], BF16),
        "onesrow": ([1, 128], BF16),
    }
    if apply_g:
        specs["lng"] = ([128, 512], F32)
    if apply_b:
        specs["lnb"] = ([128, 512], F32)
    aps = {}
    for name, (shape, dt) in specs.items():
        aps[name] = nc.dram_tensor(name, shape, dt, kind="ExternalInput").ap()
    aps["y"] = nc.dram_tensor("y", [n_tok, DIM], BF16,
                              kind="ExternalOutput").ap()

    with tile.TileContext(nc) as tc:
        with ExitStack() as ctx:
            build_core_program(ctx, tc, aps, n_tok, apply_g, apply_b)
    nc.compile()
    return nc


def _run(inputs, trace=False, **spmd_kwargs):
    from concourse.bass_utils import run_bass_kernel_spmd

    shared, per_core, apply_g, apply_b = make_host_inputs(
        inputs["x"], inputs["ln_g"], inputs["ln_b"], inputs["w_qk"],
        inputs["g4"], inputs["b4"], inputs["g2"], inputs["b2"],
        inputs["w_hidden"], inputs["w_gate"], inputs["w_out"])

    nc = build_bass(T_CORE, apply_g, apply_b)

    in_maps = [{**shared, **pc} for pc in per_core]
    res = run_bass_kernel_spmd(nc, in_maps, core_ids=list(range(N_CORES)),
                               trace=trace, **spmd_kwargs)

    y = np.empty((B, SEQ, DIM), np.float32)
    n_half = SEQ // T_CORE
    for core in range(N_CORES):
        b = core // n_half
        h = core % n_half
        y[b, h * T_CORE:(h + 1) * T_CORE] = np.asarray(
            res.results[core]["y"]).astype(np.float32)
    return y, res


def kernel(**inputs):
    return _run(inputs)[0]
